# revision 21
# baseline (speedup 1.0000x reference)
"""Multi-head attention (B=8, S=2048, E=1024, H=16, D=64) on 8 TRN2 NeuronCores.

Sharding: data parallel over batch — core b computes batch b end to end.

Per-core device kernel (all matmuls fp16, fp32 accumulation):
  scores^T[j, i] = K^T_tile.T @ Q^T          (contraction over d=64, PE, heads
                                              row-paired via tile_position)
  expS = exp(scores / 8)                     (ScalarE, PSUM -> SBUF fp16)
  acc[65, i]    = [V | 1].T @ expS           (contraction over j, PE; row 64 = softmax sums)
  concatT[e, i] = acc[0:64] * (1 / acc[64])  (DVE + gpsimd partition_broadcast)
  out[i, o]     = concatT_chunk.T @ W_out^T_chunk + b_out   (PE, natural layout)

The emission is software-pipelined: attn@V matmuls lag the exp stage by one
j-group so the PE never stalls at a group/pair boundary, and the output
projection for i-chunk ii is emitted at head-pair boundaries of chunk ii+1,
borrowing the just-freed accumulator PSUM slot.

Host does layout prep only: head-transposes Q/K to [H, D, S], appends the ones
column to V, transposes W_out, casts to fp16, and scatters/gathers per batch.
"""

import sys

if "/opt/trn_rl_repo" not in sys.path:
    sys.path.insert(0, "/opt/trn_rl_repo")

from contextlib import ExitStack

import numpy as np

B, S, E, H, D = 8, 2048, 1024, 16, 64
P = 128            # partitions
IC = 512           # query-position chunk per inner loop
NI = S // IC       # 4 i-chunks
NJ = S // P        # 16 key-position tiles
NK = E // P        # 8 contraction chunks in the output projection
NO = 2             # e_out chunks of 512
NP_ = H // 2       # 8 head pairs
SCALE = 1.0 / 8.0  # 1/sqrt(D)
# j-group sizes per exp() batch; each group's scores live in a 3-bank PSUM tile
# (two rotating slots). Layout chosen empirically against the cost model.
J_GROUPS = [(0, 3), (3, 3), (6, 3), (9, 3), (12, 2), (14, 2)]

_NC_CACHE = {}


def _build_nc():
    import concourse.mybir as mybir
    import concourse.tile as tile
    from concourse import bacc

    f32 = mybir.dt.float32
    f16 = mybir.dt.float16
    Exp = mybir.ActivationFunctionType.Exp
    mult = mybir.AluOpType.mult
    add = mybir.AluOpType.add

    nc = bacc.Bacc(
        "TRN2",
        target_bir_lowering=False,
        debug=False,
        enable_asserts=False,
        num_devices=8,
    )

    qt_d = nc.dram_tensor("qt", [H, D, S], f16, kind="ExternalInput")
    kt_d = nc.dram_tensor("kt", [H, D, S], f16, kind="ExternalInput")
    # [jt, p, hp, 130]: per s-tile row tile, per partition, per head pair:
    # [v_even(64) | 1 | v_odd(64) | 1] — the ones columns feed the softmax sums
    vt_d = nc.dram_tensor("vt", [NJ, P, NP_, 130], f16, kind="ExternalInput")
    wt_d = nc.dram_tensor("wt", [E, E], f16, kind="ExternalInput")
    bi_d = nc.dram_tensor("bias", [1, E], f32, kind="ExternalInput")
    out_d = nc.dram_tensor("out", [S, E], f32, kind="ExternalOutput")

    with tile.TileContext(nc) as tc, ExitStack() as ctx:
        const = ctx.enter_context(tc.tile_pool(name="const", bufs=1))
        qpool = ctx.enter_context(tc.tile_pool(name="qpool", bufs=2))
        epool = ctx.enter_context(tc.tile_pool(name="epool", bufs=6))
        npool = ctx.enter_context(tc.tile_pool(name="npool", bufs=4))
        opool = ctx.enter_context(tc.tile_pool(name="opool", bufs=3))
        spool = ctx.enter_context(tc.tile_pool(name="spool", bufs=2, space="PSUM"))
        apool = ctx.enter_context(tc.tile_pool(name="apool", bufs=2, space="PSUM"))

        # --- persistent tiles, DMAs chunked in first-use order ---------------
        kt_all = const.tile([P, NP_, S], f16)
        vt_all = const.tile([P, NJ, NP_, 130], f16)
        wt_all = const.tile([P, NK, E], f16)
        concatT = const.tile([P, NP_, S], f16)
        bias_row = const.tile([1, E], f32)
        bias_bc = const.tile([P, E], f32)

        kt_r = kt_d.ap().rearrange("(hp hh) d s -> (hh d) hp s", hh=2)
        qt_r = qt_d.ap().rearrange("(hp hh) d s -> (hh d) hp s", hh=2)
        vt_r = vt_d.ap().rearrange("jt p hp e -> p jt hp e")

        qt_tiles = {}

        def load_qt(ii):
            t = qpool.tile([P, NP_, IC], f16)
            isl = slice(ii * IC, (ii + 1) * IC)
            for p in range(NP_):
                nc.sync.dma_start(t[:, p, :], qt_r[:, p, isl])
            qt_tiles[ii] = t

        # first pair's operands first so compute starts within a few µs
        nc.sync.dma_start(kt_all[:, 0, :], kt_r[:, 0, :])
        load_qt(0)
        nc.sync.dma_start(vt_all[:, :, 0, :], vt_r[:, :, 0, :])
        for p in range(1, NP_):
            nc.sync.dma_start(kt_all[:, p, :], kt_r[:, p, :])
            nc.sync.dma_start(vt_all[:, :, p, :], vt_r[:, :, p, :])
        nc.sync.dma_start(wt_all[:], wt_d.ap().rearrange("(ko ki) o -> ki ko o", ki=P))
        nc.sync.dma_start(bias_row[:], bi_d.ap())
        nc.gpsimd.partition_broadcast(bias_bc[:], bias_row[:])

        # --- pipelined emission ----------------------------------------------
        # one "task" = (ii, p, group): QK pair + exp pair; AV lags one task.
        pending_av = None  # (ii, p, j0, g, exA, exB, accA, accB)
        accs = {}          # pair accumulators keyed (ii, p)
        pending_proj = []  # deferred projection tasks from finished i-chunks

        def emit_av(task):
            ii, p, j0, g, exA, exB = task
            accA, accB = accs[(ii, p)]
            for t in range(g):
                jt = j0 + t
                nc.tensor.matmul(
                    accA[:],
                    vt_all[:, jt, p, 0:65],
                    exA[:, t, :],
                    start=(jt == 0),
                    stop=(jt == NJ - 1),
                )
                nc.tensor.matmul(
                    accB[:],
                    vt_all[:, jt, p, 65:130],
                    exB[:, t, :],
                    start=(jt == 0),
                    stop=(jt == NJ - 1),
                )

        def emit_norm(ii, p):
            accA, accB = accs.pop((ii, p))
            isl = slice(ii * IC, (ii + 1) * IC)
            for hb, acc in ((0, accA), (64, accB)):
                rc = npool.tile([1, IC], f32, tag="recip")
                nc.vector.reciprocal(rc[:], acc[64:65, :])
                bc = npool.tile([64, IC], f32, tag="bcast")
                nc.gpsimd.partition_broadcast(bc[:], rc[:])
                nc.vector.tensor_tensor(
                    concatT[hb : hb + 64, p, isl], acc[0:64, :], bc[:], mult
                )

        def emit_proj(ii, it, o):
            i0 = ii * IC + it * P
            osl = slice(o * 512, (o + 1) * 512)
            pp = apool.tile([P, 512], f32, tag="acc")
            for k in range(NK):
                nc.tensor.matmul(
                    pp[:],
                    concatT[:, k, i0 : i0 + P],
                    wt_all[:, k, osl],
                    start=(k == 0),
                    stop=(k == NK - 1),
                )
            ob = opool.tile([P, 512], f32)
            nc.vector.tensor_tensor(ob[:], pp[:], bias_bc[:, osl], add)
            nc.sync.dma_start(out_d.ap()[i0 : i0 + P, osl], ob[:])

        for ii in range(NI):
            if ii + 1 < NI:
                load_qt(ii + 1)
            qt_ii = qt_tiles.pop(ii)
            for p in range(NP_):
                accs[(ii, p)] = (
                    apool.tile([65, IC], f32, tag="acc", name=f"accA_{ii}_{p}"),
                    apool.tile([65, IC], f32, tag="acc", name=f"accB_{ii}_{p}"),
                )
                sides = (0, 64)
                for gi, (j0, g) in enumerate(J_GROUPS):
                    sc = {}
                    for hb in sides:
                        sc[hb] = spool.tile([P, 3, IC], f32, tag="sc",
                                            name=f"sc{hb}_{p}_{gi}")
                    for t in range(g):
                        jt = j0 + t
                        jsl = slice(jt * P, (jt + 1) * P)
                        for hb in sides:
                            nc.tensor.matmul(
                                sc[hb][:, t, :],
                                kt_all[hb : hb + 64, p, jsl],
                                qt_ii[hb : hb + 64, p, :],
                                start=True, stop=True,
                            )
                    ex = {}
                    for hb in sides:
                        ex[hb] = epool.tile([P, 3, IC], f16, tag="ex",
                                            name=f"ex{hb}_{p}_{gi}")
                        nc.scalar.activation(
                            ex[hb][:, :g, :], sc[hb][:, :g, :], Exp, scale=SCALE
                        )
                    exA, exB = ex[0], ex[64]
                    if pending_av is not None:
                        task = pending_av
                        emit_av(task)
                        if task[2] + task[3] == NJ:  # pair's last group flushed
                            emit_norm(task[0], task[1])
                            if pending_proj:
                                emit_proj(*pending_proj.pop(0))
                    pending_av = (ii, p, j0, g, exA, exB)
            pending_proj.extend((ii, it, o) for it in range(IC // P) for o in range(NO))

        # flush
        task = pending_av
        emit_av(task)
        emit_norm(task[0], task[1])
        while pending_proj:
            emit_proj(*pending_proj.pop(0))

    nc.compile()
    return nc


def get_nc():
    if "nc" not in _NC_CACHE:
        _NC_CACHE["nc"] = _build_nc()
    return _NC_CACHE["nc"]


def make_in_maps(values, keys, queries, W_out, b_out):
    f16 = np.float16
    q = np.ascontiguousarray(
        np.asarray(queries, dtype=np.float32)
        .astype(f16)
        .reshape(B, S, H, D)
        .transpose(0, 2, 3, 1)
    )  # [B, H, D, S]
    k = np.ascontiguousarray(
        np.asarray(keys, dtype=np.float32)
        .astype(f16)
        .reshape(B, S, H, D)
        .transpose(0, 2, 3, 1)
    )
    v = np.asarray(values, dtype=np.float32).reshape(B, S, H, D)
    vt = np.empty((B, S, H, D + 1), dtype=f16)
    vt[..., :D] = v.astype(f16)
    vt[..., D] = np.float32(1.0)
    # [B, S, H, 65] -> [B, jt, p, hp, 130]
    vt = vt.reshape(B, NJ, P, NP_, 130)
    wt = np.ascontiguousarray(np.asarray(W_out, dtype=np.float32).T).astype(f16)
    bias = np.ascontiguousarray(np.asarray(b_out, dtype=np.float32).reshape(1, E))
    return [
        {"qt": q[b], "kt": k[b], "vt": vt[b], "wt": wt, "bias": bias}
        for b in range(B)
    ]


def kernel(values, keys, queries, W_out, b_out):
    from concourse.bass_utils import run_bass_kernel_spmd

    nc = get_nc()
    in_maps = make_in_maps(values, keys, queries, W_out, b_out)
    res = run_bass_kernel_spmd(nc, in_maps, core_ids=list(range(8)))
    out = np.stack([res.results[b]["out"] for b in range(B)], axis=0)
    return np.ascontiguousarray(out.astype(np.float32))
